# Initial kernel scaffold
#
"""Trainium2 Bass kernel for nn_CCDecoder: batched 30-step bicycle-model rollout.

Contract: kernel(z, init_state) -> [B, 30, 4] float32, with B = 2097152.
Data-parallel across 8 NeuronCores (B/8 rows each, no communication).

Per-element math (reference):
    steering = clip_by_tensor(0.5*z1, last_st - 0.012, last_st + 0.012); clip +-0.5
    a = clip(2.5*z0, +-2.5); tan_beta = tan(steering); k = tan_beta*DT/2.5; c = a*DT
    scan over t: v' = clip(v + c, 0, 10); psi' = psi + k*v;
                 x' = x + DT*v'*cos(psi'); y' = y + DT*v'*sin(psi')

Kernel structure per tile of 128xJ elements (each element's 30 steps laid out
contiguously along the free dim, FD = J*30):
  - v series has a closed form: v_{t+1} = clip(v1 + t*c, 0, 10)  (monotone)
  - psi/x/y cumulative sums are single tensor_tensor_scan instructions with a
    0/1 mask resetting the running state at each element boundary
  - sin/cos via half-angle identities on a per-element folded psi0 so the
    ScalarE Sin argument stays within its [-pi, pi] domain
  - the staging tile holds the exact DRAM image ([x,y,psi,v] x 30 per element)
    so the output DMA is one contiguous transfer per tile
"""

from contextlib import ExitStack

import numpy as np

import concourse.bass as bass
import concourse.mybir as mybir
import concourse.tile as tile
from concourse.bass_utils import run_bass_kernel_spmd

F32 = mybir.dt.float32
ALU = mybir.AluOpType
ACTF = mybir.ActivationFunctionType

DT = 0.03
T = 30
D_STEER = 0.4 * DT  # 0.012
PI = float(np.pi)
HALF_PI = float(np.pi / 2)
TWO_PI = float(2 * np.pi)

P = 128
N_CORES = 8
B_TOTAL = 2097152
B_CORE = B_TOTAL // N_CORES  # 262144


def build_kernel(b_core=B_CORE, j=64):
    """Build the per-core Bass program. Same program runs SPMD on all cores."""
    fd = j * T
    nt = b_core // (P * j)
    assert nt * P * j == b_core

    nc = bass.Bass()
    z = nc.dram_tensor("z", [b_core, 2], F32, kind="ExternalInput")
    s = nc.dram_tensor("init_state", [b_core, 6], F32, kind="ExternalInput")
    out = nc.dram_tensor("out", [b_core, 4 * T], F32, kind="ExternalOutput")

    zr = z.rearrange("(n p j) c -> n p (j c)", p=P, j=j)
    sr = s.rearrange("(n p j) c -> n p (j c)", p=P, j=j)
    outr = out.rearrange("(n p j) c -> n p (j c)", p=P, j=j)

    with tile.TileContext(nc) as tc, ExitStack() as ctx:
        const_pool = ctx.enter_context(tc.tile_pool(name="const", bufs=1))
        io_pool = ctx.enter_context(tc.tile_pool(name="io", bufs=3))
        small_pool = ctx.enter_context(tc.tile_pool(name="small", bufs=3))
        big_pool = ctx.enter_context(tc.tile_pool(name="big", bufs=10))
        stage_pool = ctx.enter_context(tc.tile_pool(name="stage", bufs=2))

        # Constants (built once)
        mask = const_pool.tile([P, fd], F32)
        nc.vector.memset(mask[:], 1.0)
        mask3 = mask[:].rearrange("p (j t) -> p j t", t=T)
        nc.vector.memset(mask3[:, :, 0], 0.0)
        iota_f = const_pool.tile([P, T], F32)
        nc.gpsimd.iota(
            iota_f[:], [[1, T]], channel_multiplier=0,
            allow_small_or_imprecise_dtypes=True,
        )
        zeros = const_pool.tile([P, j], F32)
        nc.vector.memset(zeros[:], 0.0)

        iota_b = iota_f[:].unsqueeze(1).broadcast_to([P, j, T])

        def small(name):
            return small_pool.tile([P, j], F32, tag=name)

        def big():
            return big_pool.tile([P, fd], F32, tag="big")

        for ti in range(nt):
            z_t = io_pool.tile([P, 2 * j], F32, tag="zt")
            nc.sync.dma_start(z_t[:], zr[ti])
            s_t = io_pool.tile([P, 6 * j], F32, tag="st")
            nc.sync.dma_start(s_t[:], sr[ti])

            zv = z_t[:].rearrange("p (j c) -> p j c", c=2)
            sv = s_t[:].rearrange("p (j c) -> p j c", c=6)
            x0, y0, psi0, v0, last = (
                sv[:, :, 0], sv[:, :, 1], sv[:, :, 2], sv[:, :, 3], sv[:, :, 5]
            )

            # ---- per-element preamble (FD = j) ----
            ped = small("ped")
            nc.vector.tensor_scalar(ped[:], zv[:, :, 0], 2.5, 2.5, ALU.mult, ALU.min)
            c = small("c")  # c = a_t * DT
            nc.vector.tensor_scalar(c[:], ped[:], -2.5, DT, ALU.max, ALU.mult)

            tmin = small("tmin")
            nc.vector.tensor_scalar(tmin[:], last, D_STEER, None, ALU.subtract)
            tmax = small("tmax")
            nc.vector.tensor_scalar(tmax[:], last, D_STEER, None, ALU.add)
            st_r = small("st_r")
            nc.vector.tensor_scalar(st_r[:], zv[:, :, 1], 0.5, None, ALU.mult)
            mx = small("mx")
            nc.vector.tensor_tensor(mx[:], st_r[:], tmin[:], ALU.max)
            meq = small("meq")
            nc.vector.tensor_tensor(meq[:], st_r[:], tmin[:], ALU.is_equal)
            # clip_by_tensor quirk: where steering == tmin the result is 0
            nc.vector.copy_predicated(mx[:], meq[:], zeros[:])
            beta = small("beta")
            nc.vector.tensor_tensor(beta[:], mx[:], tmax[:], ALU.min)
            nc.vector.tensor_scalar(beta[:], beta[:], -0.5, 0.5, ALU.max, ALU.min)

            sb = small("sb")
            nc.scalar.activation(sb[:], beta[:], ACTF.Sin)
            cb = small("cb")
            nc.scalar.activation(cb[:], beta[:], ACTF.Sin, bias=HALF_PI)
            tanb = small("tanb")
            nc.vector.tensor_tensor(tanb[:], sb[:], cb[:], ALU.divide)
            k = small("k")
            nc.vector.tensor_scalar(k[:], tanb[:], DT / 2.5, None, ALU.mult)

            v1 = small("v1")
            nc.vector.tensor_tensor(v1[:], v0, c[:], ALU.add)
            nc.vector.tensor_scalar(v1[:], v1[:], 0.0, 10.0, ALU.max, ALU.min)

            # fold psi0 into [-pi, pi] (|psi0| < 3*pi for randn inputs)
            mgt = small("mgt")
            nc.vector.tensor_scalar(mgt[:], psi0, PI, None, ALU.is_gt)
            mlt = small("mlt")
            nc.vector.tensor_scalar(mlt[:], psi0, -PI, None, ALU.is_lt)
            dd = small("dd")
            nc.vector.tensor_tensor(dd[:], mlt[:], mgt[:], ALU.subtract)
            psi0f = small("psi0f")
            nc.vector.scalar_tensor_tensor(
                psi0f[:], dd[:], TWO_PI, psi0, ALU.mult, ALU.add
            )
            npi = small("npi")  # psi0f - psi0 = 2*pi*n
            nc.vector.tensor_tensor(npi[:], psi0f[:], psi0, ALU.subtract)

            c_b = c[:].unsqueeze(2).broadcast_to([P, j, T])
            v1_b = v1[:].unsqueeze(2).broadcast_to([P, j, T])
            k_b = k[:].unsqueeze(2).broadcast_to([P, j, T])
            npi_b = npi[:].unsqueeze(2).broadcast_to([P, j, T])

            # ---- series phase (FD = j*T) ----
            staging = stage_pool.tile([P, 2 + 4 * fd], F32, tag="stg")
            nc.vector.memset(staging[:, 0:2], 0.0)
            stg4 = staging[:, 2:].rearrange("p (j t c) -> p j t c", t=T, c=4)
            stg_x, stg_y, stg_psi, stg_v = (
                stg4[:, :, :, 0], stg4[:, :, :, 1], stg4[:, :, :, 2], stg4[:, :, :, 3]
            )
            # v at (j, t-1); at t=0 reads garbage, fixed via B slot-0 overwrite
            vshift = (
                staging[:, 1 : 1 + 4 * fd]
                .rearrange("p (j t c) -> p j t c", t=T, c=4)[:, :, :, 0]
            )

            # v series: v_{t+1} = clip(v1 + t*c, 0, 10) -> staging
            vm = big()
            vm3 = vm[:].rearrange("p (j t) -> p j t", t=T)
            nc.gpsimd.tensor_tensor(vm3, iota_b, c_b, ALU.mult)
            vl = big()
            vl3 = vl[:].rearrange("p (j t) -> p j t", t=T)
            nc.gpsimd.tensor_tensor(vl3, vm3, v1_b, ALU.add)
            nc.gpsimd.tensor_scalar(stg_v, vl3, 0.0, 10.0, ALU.max, ALU.min)

            # psi series (folded space): scan of B, B[t] = k*v_t, slot0 = psi0f + k*v0
            B = big()
            B3 = B[:].rearrange("p (j t) -> p j t", t=T)
            nc.vector.tensor_tensor(B3, k_b, vshift, ALU.mult)
            kv0 = small("kv0")
            nc.vector.tensor_tensor(kv0[:], k[:], v0, ALU.mult)
            nc.vector.tensor_tensor(B3[:, :, 0], kv0[:], psi0f[:], ALU.add)
            ptrig = big()
            nc.vector.tensor_tensor_scan(
                ptrig[:], mask[:], B[:], 0.0, ALU.mult, ALU.add
            )
            ptrig3 = ptrig[:].rearrange("p (j t) -> p j t", t=T)
            # unfold: raw psi = ptrig - 2*pi*n -> staging
            nc.vector.tensor_tensor(stg_psi, ptrig3, npi_b, ALU.subtract)

            # trig via half-angle: s2 = sin(psi/2), c2 = cos(psi/2)
            s2 = big()
            nc.scalar.activation(s2[:], ptrig[:], ACTF.Sin, scale=0.5)
            c2 = big()
            nc.scalar.activation(c2[:], ptrig[:], ACTF.Sin, bias=HALF_PI, scale=0.5)
            sinv = big()
            nc.vector.scalar_tensor_tensor(
                sinv[:], s2[:], 2.0, c2[:], ALU.mult, ALU.mult
            )
            m2 = big()
            nc.vector.scalar_tensor_tensor(
                m2[:], s2[:], -2.0, s2[:], ALU.mult, ALU.mult
            )
            cosv = big()
            nc.vector.tensor_scalar(cosv[:], m2[:], 1.0, None, ALU.add)

            # dx = (cos*DT)*v', dy = (sin*DT)*v'; slot0 += x0/y0; then scan
            cosv3 = cosv[:].rearrange("p (j t) -> p j t", t=T)
            sinv3 = sinv[:].rearrange("p (j t) -> p j t", t=T)
            dx = big()
            dx3 = dx[:].rearrange("p (j t) -> p j t", t=T)
            nc.vector.scalar_tensor_tensor(dx3, cosv3, DT, stg_v, ALU.mult, ALU.mult)
            dy = big()
            dy3 = dy[:].rearrange("p (j t) -> p j t", t=T)
            nc.gpsimd.scalar_tensor_tensor(dy3, sinv3, DT, stg_v, ALU.mult, ALU.mult)
            tx = small("tx")
            nc.vector.tensor_tensor(tx[:], dx3[:, :, 0], x0, ALU.add)
            nc.vector.tensor_copy(dx3[:, :, 0], tx[:])
            ty = small("ty")
            nc.vector.tensor_tensor(ty[:], dy3[:, :, 0], y0, ALU.add)
            nc.vector.tensor_copy(dy3[:, :, 0], ty[:])

            xser = big()
            nc.vector.tensor_tensor_scan(
                xser[:], mask[:], dx[:], 0.0, ALU.mult, ALU.add
            )
            yser = big()
            nc.gpsimd.tensor_tensor_scan(
                yser[:], mask[:], dy[:], 0.0, ALU.mult, ALU.add
            )
            nc.scalar.copy(stg_x, xser[:].rearrange("p (j t) -> p j t", t=T))
            nc.scalar.copy(stg_y, yser[:].rearrange("p (j t) -> p j t", t=T))

            nc.sync.dma_start(outr[ti], staging[:, 2:])

    return nc


def kernel(z, init_state):
    z = np.ascontiguousarray(np.asarray(z, dtype=np.float32))
    s = np.ascontiguousarray(np.asarray(init_state, dtype=np.float32))
    assert z.shape == (B_TOTAL, 2) and s.shape == (B_TOTAL, 6)

    nc = build_kernel()
    zs = z.reshape(N_CORES, B_CORE, 2)
    ss = s.reshape(N_CORES, B_CORE, 6)
    in_maps = [
        {"z": np.ascontiguousarray(zs[i]), "init_state": np.ascontiguousarray(ss[i])}
        for i in range(N_CORES)
    ]
    res = run_bass_kernel_spmd(nc, in_maps, core_ids=list(range(N_CORES)))
    parts = [res.results[i]["out"].reshape(B_CORE, T, 4) for i in range(N_CORES)]
    return np.concatenate(parts, axis=0)


if __name__ == "__main__":
    rng = np.random.default_rng(0)
    zz = rng.standard_normal((B_TOTAL, 2), dtype=np.float32)
    si = rng.standard_normal((B_TOTAL, 6), dtype=np.float32)
    o = kernel(zz, si)
    print(o.shape, o.dtype)


# revision 20
# speedup vs baseline: 23.6948x; 23.6948x over previous
"""Trainium2 Bass kernel for nn_CCDecoder: batched 30-step bicycle-model rollout.

Contract: kernel(z, init_state) -> [B, 30, 4] float32, with B = 2097152.
Data-parallel across 8 NeuronCores (B/8 rows each, no communication).

Per-element math (reference):
    steering = clip_by_tensor(0.5*z1, last_st - 0.012, last_st + 0.012); clip +-0.5
    a = clip(2.5*z0, +-2.5); tan_beta = tan(steering); k = tan_beta*DT/2.5; c = a*DT
    scan over t: v' = clip(v + c, 0, 10); psi' = psi + k*v;
                 x' = x + DT*v'*cos(psi'); y' = y + DT*v'*sin(psi')

Kernel structure per tile of 128xJ elements (each element's 30 steps laid out
contiguously along the free dim, FD = J*30):
  - v series has a closed form: v_{t+1} = clip(v1 + t*c, 0, 10)  (monotone)
  - psi/x/y cumulative sums are single tensor_tensor_scan instructions with a
    0/1 mask resetting the running state at each element boundary
  - sin/cos via half-angle identities on a per-element folded psi0 so the
    ScalarE Sin argument stays within its [-pi, pi] domain
  - the staging tile holds the exact DRAM image ([x,y,psi,v] x 30 per element)
    so the output DMA is one contiguous transfer per tile

Tile tags are segregated so each SBUF buffer has a single producer engine and
few consumer engines — the HW caps semaphore waits per instruction.
"""

from contextlib import ExitStack

import numpy as np

import concourse.bacc as bacc
import concourse.bass as bass
import concourse.mybir as mybir
import concourse.tile as tile
from concourse.bass_utils import run_bass_kernel_spmd

F32 = mybir.dt.float32
ALU = mybir.AluOpType
ACTF = mybir.ActivationFunctionType

DT = 0.03
T = 30
D_STEER = 0.4 * DT  # 0.012
PI = float(np.pi)
HALF_PI = float(np.pi / 2)
TWO_PI = float(2 * np.pi)

P = 128
N_CORES = 8
B_TOTAL = 2097152
B_CORE = B_TOTAL // N_CORES  # 262144


def build_kernel(b_core=B_CORE, j=32, reps=1):
    """Build the per-core Bass program. Same program runs SPMD on all cores.

    reps > 1 repeats the whole pass (same inputs/outputs) for timing: the
    wall-clock delta between reps=1 and reps=N isolates on-device time from
    host<->device transfer time.
    """
    fd = j * T
    nt = b_core // (P * j)
    assert nt * P * j == b_core

    nc = bacc.Bacc()
    z = nc.dram_tensor("z", [b_core, 2], F32, kind="ExternalInput")
    s = nc.dram_tensor("init_state", [b_core, 6], F32, kind="ExternalInput")
    out = nc.dram_tensor("out", [b_core, 4 * T], F32, kind="ExternalOutput")

    zr = z.rearrange("(n p j) c -> n p (j c)", p=P, j=j)
    sr = s.rearrange("(n p j) c -> n p (j c)", p=P, j=j)
    outr = out.rearrange("(n p j) c -> n p (j c)", p=P, j=j)

    # Register activation-bias constants (same mechanism Bass uses at init).
    for val in (HALF_PI,):
        t = nc.alloc_sbuf_tensor(f"const-f32-{val}", [128, 1], F32)
        nc.gpsimd.memset(t.ap(), val)
        nc.const_aps.aps[(F32, val)] = t.ap()
    nc.all_engine_barrier()

    with tile.TileContext(nc) as tc, ExitStack() as ctx:
        const_pool = ctx.enter_context(tc.tile_pool(name="const", bufs=1))
        io_pool = ctx.enter_context(tc.tile_pool(name="io", bufs=3))
        small_pool = ctx.enter_context(tc.tile_pool(name="small", bufs=3))
        big_pool = ctx.enter_context(tc.tile_pool(name="big", bufs=2))
        stage_pool = ctx.enter_context(tc.tile_pool(name="stage", bufs=2))

        # Constants (built once)
        mask = const_pool.tile([P, fd], F32)
        nc.vector.memset(mask[:], 1.0)
        mask3 = mask[:].rearrange("p (j t) -> p j t", t=T)
        nc.vector.memset(mask3[:, :, 0], 0.0)
        iota_f = const_pool.tile([P, T], F32)
        nc.gpsimd.iota(
            iota_f[:], [[1, T]], channel_multiplier=0,
            allow_small_or_imprecise_dtypes=True,
        )

        iota_b = iota_f[:].unsqueeze(1).broadcast_to([P, j, T])

        def small(name):
            return small_pool.tile([P, j], F32, tag=name, name=name)

        def big(name, tag, bufs=2):
            return big_pool.tile([P, fd], F32, tag=tag, name=name, bufs=bufs)

        for ti in [i for _ in range(reps) for i in range(nt)]:
            z_t = io_pool.tile([P, 2 * j], F32, tag="zt", name="zt")
            nc.sync.dma_start(z_t[:], zr[ti])
            s_t = io_pool.tile([P, 6 * j], F32, tag="st", name="st")
            nc.sync.dma_start(s_t[:], sr[ti])

            zv = z_t[:].rearrange("p (j c) -> p j c", c=2)
            sv = s_t[:].rearrange("p (j c) -> p j c", c=6)
            x0, y0, psi0, v0, last = (
                sv[:, :, 0], sv[:, :, 1], sv[:, :, 2], sv[:, :, 3], sv[:, :, 5]
            )

            # ---- per-element preamble (FD = j), all on VectorE except sins ----
            ped = small("ped")
            nc.vector.tensor_scalar(ped[:], zv[:, :, 0], 2.5, 2.5, ALU.mult, ALU.min)
            c = small("c")  # c = a_t * DT
            nc.vector.tensor_scalar(c[:], ped[:], -2.5, DT, ALU.max, ALU.mult)

            tmin = small("tmin")
            nc.vector.tensor_scalar(tmin[:], last, D_STEER, None, ALU.subtract)
            tmax = small("tmax")
            nc.vector.tensor_scalar(tmax[:], last, D_STEER, None, ALU.add)
            st_r = small("st_r")
            nc.vector.tensor_scalar(st_r[:], zv[:, :, 1], 0.5, None, ALU.mult)
            mx = small("mx")
            nc.vector.tensor_tensor(mx[:], st_r[:], tmin[:], ALU.max)
            neq = small("neq")
            nc.vector.tensor_tensor(neq[:], st_r[:], tmin[:], ALU.not_equal)
            # clip_by_tensor quirk: where steering == tmin the result is 0
            nc.vector.tensor_tensor(mx[:], mx[:], neq[:], ALU.mult)
            beta = small("beta")
            nc.vector.tensor_tensor(beta[:], mx[:], tmax[:], ALU.min)
            nc.vector.tensor_scalar(beta[:], beta[:], -0.5, 0.5, ALU.max, ALU.min)

            sb = small("sb")
            nc.scalar.activation(sb[:], beta[:], ACTF.Sin)
            cb = small("cb")
            nc.scalar.activation(cb[:], beta[:], ACTF.Sin, bias=HALF_PI)
            rc = small("rc")
            nc.vector.reciprocal(rc[:], cb[:])
            tanb = small("tanb")
            nc.vector.tensor_tensor(tanb[:], sb[:], rc[:], ALU.mult)
            k = small("k")
            nc.vector.tensor_scalar(k[:], tanb[:], DT / 2.5, None, ALU.mult)

            v1 = small("v1")
            nc.vector.tensor_tensor(v1[:], v0, c[:], ALU.add)
            nc.vector.tensor_scalar(v1[:], v1[:], 0.0, 10.0, ALU.max, ALU.min)

            # fold psi0 into [-pi, pi] (|psi0| < 3*pi for randn inputs)
            mgt = small("mgt")
            nc.vector.tensor_scalar(mgt[:], psi0, PI, None, ALU.is_gt)
            mlt = small("mlt")
            nc.vector.tensor_scalar(mlt[:], psi0, -PI, None, ALU.is_lt)
            dd = small("dd")
            nc.vector.tensor_tensor(dd[:], mlt[:], mgt[:], ALU.subtract)
            psi0f = small("psi0f")
            nc.vector.scalar_tensor_tensor(
                psi0f[:], dd[:], TWO_PI, psi0, ALU.mult, ALU.add
            )
            npi = small("npi")  # psi0f - psi0 = 2*pi*n
            nc.vector.tensor_tensor(npi[:], psi0f[:], psi0, ALU.subtract)

            c_b = c[:].unsqueeze(2).broadcast_to([P, j, T])
            v1_b = v1[:].unsqueeze(2).broadcast_to([P, j, T])
            k_b = k[:].unsqueeze(2).broadcast_to([P, j, T])
            npi_b = npi[:].unsqueeze(2).broadcast_to([P, j, T])

            # ---- series phase (FD = j*T) ----
            staging = stage_pool.tile([P, 4 * fd], F32, tag="stg", name="stg")
            stg4 = staging[:].rearrange("p (j t c) -> p j t c", t=T, c=4)
            stg_x, stg_y, stg_psi, stg_v = (
                stg4[:, :, :, 0], stg4[:, :, :, 1], stg4[:, :, :, 2], stg4[:, :, :, 3]
            )

            # v series: v_{t+1} = clip(v1 + t*c, 0, 10), kept contiguous in a
            # front-padded tile so the psi scan can read v at (j, t-1) via a
            # shifted view (garbage at t=0 is overwritten by the slot-0 fix).
            # Pool engine owns this chain.
            vm = big("vm", "pv", bufs=4)
            vm3 = vm[:].rearrange("p (j t) -> p j t", t=T)
            nc.gpsimd.tensor_tensor(vm3, iota_b, c_b, ALU.mult)
            vl = big("vl", "pv", bufs=4)
            vl3 = vl[:].rearrange("p (j t) -> p j t", t=T)
            nc.gpsimd.tensor_tensor(vl3, vm3, v1_b, ALU.add)
            vcp = big_pool.tile([P, 2 + fd], F32, tag="vcp", name="vcp", bufs=2)
            nc.gpsimd.memset(vcp[:, 0:2], 0.0)
            vc3 = vcp[:, 2:].rearrange("p (j t) -> p j t", t=T)
            nc.gpsimd.tensor_scalar(vc3, vl3, 0.0, 10.0, ALU.max, ALU.min)
            nc.scalar.copy(stg_v, vc3)
            vshift = vcp[:, 1 : 1 + fd].rearrange("p (j t) -> p j t", t=T)

            # psi series (folded space): scan of B; B[t] = k*v_t, slot0 = psi0f + k*v0
            B = big("B", "vb")
            B3 = B[:].rearrange("p (j t) -> p j t", t=T)
            nc.vector.tensor_tensor(B3, k_b, vshift, ALU.mult)
            kv0 = small("kv0")
            nc.vector.tensor_tensor(kv0[:], k[:], v0, ALU.mult)
            nc.vector.tensor_tensor(B3[:, :, 0], kv0[:], psi0f[:], ALU.add)
            ptrig = big("ptrig", "vsc", bufs=4)
            nc.vector.tensor_tensor_scan(
                ptrig[:], mask[:], B[:], 0.0, ALU.mult, ALU.add
            )
            ptrig3 = ptrig[:].rearrange("p (j t) -> p j t", t=T)
            # unfold: raw psi = ptrig - 2*pi*n -> staging (Pool engine)
            nc.gpsimd.tensor_tensor(stg_psi, ptrig3, npi_b, ALU.subtract)

            # trig via half-angle: s2 = sin(psi/2), c2 = cos(psi/2)  (ScalarE)
            s2 = big("s2", "s2")
            nc.scalar.activation(s2[:], ptrig[:], ACTF.Sin, scale=0.5)
            c2 = big("c2", "c2")
            nc.scalar.activation(c2[:], ptrig[:], ACTF.Sin, bias=HALF_PI, scale=0.5)
            sinv = big("sinv", "sv")
            nc.vector.scalar_tensor_tensor(
                sinv[:], s2[:], 2.0, c2[:], ALU.mult, ALU.mult
            )
            m2 = big("m2", "vt", bufs=6)
            nc.vector.scalar_tensor_tensor(
                m2[:], s2[:], -2.0, s2[:], ALU.mult, ALU.mult
            )
            cosv = big("cosv", "vt", bufs=6)
            nc.vector.tensor_scalar(cosv[:], m2[:], 1.0, None, ALU.add)

            # ws = DT*v' (Pool); dx = cos*ws (VectorE chain), dy = sin*ws (Pool chain)
            ws = big("ws", "pw", bufs=2)
            ws3 = ws[:].rearrange("p (j t) -> p j t", t=T)
            nc.gpsimd.tensor_scalar(ws[:], vcp[:, 2:], DT, None, ALU.mult)
            cosv3 = cosv[:].rearrange("p (j t) -> p j t", t=T)
            sinv3 = sinv[:].rearrange("p (j t) -> p j t", t=T)
            dx = big("dx", "vt", bufs=6)
            dx3 = dx[:].rearrange("p (j t) -> p j t", t=T)
            nc.vector.tensor_tensor(dx3, cosv3, ws3, ALU.mult)
            tx = small("tx")
            nc.vector.tensor_tensor(tx[:], dx3[:, :, 0], x0, ALU.add)
            nc.vector.tensor_copy(dx3[:, :, 0], tx[:])
            xser = big("xser", "vsc", bufs=4)
            nc.vector.tensor_tensor_scan(
                xser[:], mask[:], dx[:], 0.0, ALU.mult, ALU.add
            )
            nc.scalar.copy(stg_x, xser[:].rearrange("p (j t) -> p j t", t=T))

            dy = big("dy", "py", bufs=4)
            dy3 = dy[:].rearrange("p (j t) -> p j t", t=T)
            nc.gpsimd.tensor_tensor(dy3, sinv3, ws3, ALU.mult)
            ty = small("ty")
            nc.gpsimd.tensor_tensor(ty[:], dy3[:, :, 0], y0, ALU.add)
            nc.gpsimd.tensor_copy(dy3[:, :, 0], ty[:])
            yser = big("yser", "py", bufs=4)
            nc.vector.tensor_tensor_scan(
                yser[:], mask[:], dy[:], 0.0, ALU.mult, ALU.add
            )
            nc.scalar.copy(stg_y, yser[:].rearrange("p (j t) -> p j t", t=T))

            nc.sync.dma_start(outr[ti], staging[:])

    nc.compile()
    return nc


def kernel(z, init_state):
    z = np.ascontiguousarray(np.asarray(z, dtype=np.float32))
    s = np.ascontiguousarray(np.asarray(init_state, dtype=np.float32))
    assert z.shape == (B_TOTAL, 2) and s.shape == (B_TOTAL, 6)

    nc = build_kernel()
    zs = z.reshape(N_CORES, B_CORE, 2)
    ss = s.reshape(N_CORES, B_CORE, 6)
    in_maps = [
        {"z": np.ascontiguousarray(zs[i]), "init_state": np.ascontiguousarray(ss[i])}
        for i in range(N_CORES)
    ]
    res = run_bass_kernel_spmd(nc, in_maps, core_ids=list(range(N_CORES)))
    parts = [res.results[i]["out"].reshape(B_CORE, T, 4) for i in range(N_CORES)]
    return np.concatenate(parts, axis=0)


if __name__ == "__main__":
    rng = np.random.default_rng(0)
    zz = rng.standard_normal((B_TOTAL, 2), dtype=np.float32)
    si = rng.standard_normal((B_TOTAL, 6), dtype=np.float32)
    o = kernel(zz, si)
    print(o.shape, o.dtype)
